# revision 27
# baseline (speedup 1.0000x reference)
"""Trainium2 Bass kernel: batch-512 LSTM (H=64, D=128, T=1024) + tanh decoder.

Strategy: data-parallel over batch across 8 NeuronCores (64 rows each).
Per core, transposed-state layout: state hT/c are [H, B] tiles; gates land in
two PSUM banks ((f,i) and (o,j)); sigmoid over the (f,i) bank starts one
matmul early, tanh(j)/sigmoid(o) follow (all in one ACT table set -> single
table load). Biases ride in via an augmented ones-row on the h-side matmul
(K=65). The per-step dependency cycle measured on silicon (~1.99us/step):
add(183) tanh_c(348) h-mul(184) whb-matmul(212) sigma_fi(314) tanh_j(310)
u-mul(188) + ~30-50ns sem gaps — every op is fixed-latency dominated, and
all cheaper reorderings were measured/modeled to be neutral or worse (see
notes: NG=2 saturates ACT at ~280ns/op engine-busy; PSUM rerouting trades
ACT access savings for equal DVE penalties).

Overhead fixes that took 2.70ms -> 2.06ms:
- decoder: per-step matmuls accumulate DEC_BLK=16 steps into a small PSUM
  tile ([64,256]f32 — a [64,512] tile caused bank pressure that slowed ALL
  engines ~20%); the batched tanh is emitted at the END of a step so it sits
  AFTER tanh_c in the in-order ACT queue (ahead of the next gate sigmoid it
  costs +690ns); global-step indexing defers each chunk's final block into
  the next chunk's first step (kills +350ns at every chunk boundary).
- obss is pre-transposed on the HOST to [D, T, BL] so the per-chunk input
  load is a fat contiguous DMA instead of an element-scattered transpose.
- chunk plan [16, 32 x 31, 16]: small first chunk -> compute starts ~4x
  sooner; small last chunk -> smaller tail store.
All recurrence elementwise runs in bf16 (end-to-end rel err ~8e-3 vs f32).
"""
import sys

sys.path.insert(0, "/opt/trn_rl_repo")

import numpy as np
import ml_dtypes

import concourse.bass as bass
import concourse.bacc as bacc
import concourse.mybir as mybir
from concourse.tile import TileContext
from concourse.bass_utils import run_bass_kernel_spmd

BF16 = ml_dtypes.bfloat16
F32 = mybir.dt.float32
FB = mybir.dt.bfloat16
AF = mybir.ActivationFunctionType
OP = mybir.AluOpType

B, T, D, H, A = 512, 1024, 128, 64, 16
NCORES = 8
BL = B // NCORES  # 64 batch rows per core
TC = 32           # timesteps per chunk (small first chunk -> fast start)
DEC_BLK = 8       # timesteps per decoder PSUM bank (8*16 = 128 f32; the
                  # [64,256] tanh slightly overflowed the post-tanh_c ACT
                  # window, +115ns on ~20 steps)

C_DT = FB         # cell-state dtype (bf16 verified: end-to-end rel err ~8e-3)
NG = 1            # interleaved batch groups per core (NG=2 saturates ACT:
                  # ~280ns engine-busy per ACT op regardless of columns, and
                  # 8 gate-ACTs/step push the scalar engine to 93% -> 2.84ms)
GP_OFFLOAD = False # run m1/v4 on GpSimd to unload the vector engine


def build_nc(t_total=T):
    nc = bacc.Bacc()
    # obss arrives HOST-PRE-TRANSPOSED as [D, T, BL]: the per-chunk load is
    # then a fat contiguous DMA (4KB/partition) instead of an element-
    # scattered dma_start_transpose that kept the DMA engines busy >50% of
    # the run and contended with ACT/DVE SBUF ports.
    obss = nc.declare_dram_parameter("obss", [D, T, BL], FB, isOutput=False)
    wxif_d = nc.declare_dram_parameter("wxif", [D, 2 * H], FB, isOutput=False)
    wxjo_d = nc.declare_dram_parameter("wxjo", [D, 2 * H], FB, isOutput=False)
    whbif_d = nc.declare_dram_parameter("whbif", [H + 1, 2 * H], FB, isOutput=False)
    whbjo_d = nc.declare_dram_parameter("whbjo", [H + 1, 2 * H], FB, isOutput=False)
    decwb_d = nc.declare_dram_parameter("decwb", [H + 1, A], FB, isOutput=False)
    out = nc.declare_dram_parameter("out", [BL, T, A], F32, isOutput=True)

    with TileContext(nc) as tc:
        with (
            tc.tile_pool(name="const", bufs=1) as cpool,
            tc.tile_pool(name="state", bufs=1) as spool,
            tc.tile_pool(name="xT", bufs=2) as xpool,
            tc.tile_pool(name="stage", bufs=2) as stpool,
            tc.tile_pool(name="work", bufs=3) as wpool,
            tc.tile_pool(name="psz", bufs=2, space="PSUM") as pzpool,
            tc.tile_pool(name="psd", bufs=2, space="PSUM") as pdpool,
        ):
            # all tiles allocated 128-partition so every base partition is 0
            # (2-input DVE ops require equal input base partitions)
            wxif = cpool.tile([D, 2 * H], FB, tag="wxif")
            wxjo = cpool.tile([D, 2 * H], FB, tag="wxjo")
            whbif_t = cpool.tile([D, 2 * H], FB, tag="whbif")
            whbjo_t = cpool.tile([D, 2 * H], FB, tag="whbjo")
            decwb_t = cpool.tile([D, A], FB, tag="decwb")
            whbif = whbif_t[0 : H + 1, :]
            whbjo = whbjo_t[0 : H + 1, :]
            decwb = decwb_t[0 : H + 1, :]
            # first input chunk's DMA is hoisted AHEAD of the weight loads:
            # the Sync queue generates descriptors serially (~0.7us each), and
            # the first matmul needs chunk 0's 256KB before anything happens —
            # queueing it first starts compute ~3.5us sooner.
            FIRST_TC = 8
            xT0 = xpool.tile([D, FIRST_TC * BL], FB, tag="xT0", name="xT0")
            nc.sync.dma_start(
                xT0[:, :],
                obss[:, 0:FIRST_TC, :].rearrange("d t b -> d (t b)"),
            )
            # weight loads on the Sync queue after chunk 0's input (moving
            # them to the ACT queue measured slightly worse: 2055184 vs
            # 2051350)
            nc.sync.dma_start(wxif[:, :], wxif_d[:, :])
            nc.sync.dma_start(whbif, whbif_d[:, :])
            nc.sync.dma_start(wxjo[:, :], wxjo_d[:, :])
            nc.sync.dma_start(whbjo, whbjo_d[:, :])
            nc.sync.dma_start(decwb, decwb_d[:, :])

            BG = BL // NG  # batch rows per group
            hTs, csts = [], []
            for g in range(NG):
                hT_t = spool.tile([D, BG], FB, tag=f"hT{g}")
                cst_t = spool.tile([D, BG], C_DT, tag=f"c{g}")
                nc.vector.memset(hT_t[0:H, :], 0.0)
                nc.vector.memset(hT_t[H : H + 1, :], 1.0)
                nc.vector.memset(cst_t[0:H, :], 0.0)
                hTs.append(hT_t)
                csts.append(cst_t)

            # chunk plan: small first chunk -> first input DMA lands sooner;
            # tiny last chunks -> the only output DMA that can't overlap
            # compute (the final one) shrinks to 64KB
            chunk_plan = [8] + [32] * ((t_total - 32) // 32) + [8, 8, 8]
            assert sum(chunk_plan) == t_total
            dec_state = {}
            chunk_recs = []  # (t0, tc, stage tile) in emission order

            def stage_of(gt):
                for t0, tc_, stg in chunk_recs:
                    if t0 <= gt < t0 + tc_:
                        return t0, tc_, stg
                raise AssertionError(gt)

            def emit_dec(gt, g):
                # decoder matmul for GLOBAL step gt, group g:
                # out[b, A] = h @ dec_w + dec_b via the ones-row of hT
                # (batched tanh emitted separately, off the ACT critical path)
                if gt < 0:
                    return
                dcol = gt % DEC_BLK
                if dcol == 0 and g == 0:
                    psd_tile = pdpool.tile([BL, DEC_BLK * A], F32, tag="psd")
                    dec_state["psd"] = psd_tile
                psd = dec_state["psd"]
                nc.tensor.matmul(
                    psd[g * BG : (g + 1) * BG, dcol * A : (dcol + 1) * A],
                    hTs[g][0 : H + 1, :], decwb, start=True, stop=True,
                )

            def emit_dec_tanh(gt):
                # batched decoder tanh for the DEC_BLK block ending at global
                # step gt. Emitted at the END of a step's ACT queue so it
                # fills the ACT idle window after tanh(c) instead of delaying
                # the next step's gate sigmoid (+690ns per occurrence). Global
                # indexing defers a chunk's final block into the next chunk's
                # first step (fixes a +350ns stall at every chunk boundary);
                # the chunk's output DMA is emitted right after it.
                if gt < 0 or (gt % DEC_BLK) != DEC_BLK - 1:
                    return
                t0, tc_, stg = stage_of(gt)
                blk0 = ((gt - t0) // DEC_BLK) * DEC_BLK
                nc.scalar.activation(
                    stg[:, blk0 * A : (blk0 + DEC_BLK) * A],
                    dec_state["psd"][:, :], AF.Tanh,
                )
                if gt == t0 + tc_ - 1:
                    nc.sync.dma_start(out[:, t0 : t0 + tc_, :], stg[:, :])

            gt = 0
            for ci, TC in enumerate(chunk_plan):
                t0 = gt
                if ci == 0:
                    assert TC == FIRST_TC
                    xT = xT0
                else:
                    xT = xpool.tile([D, TC * BL], FB, tag="xT")
                    nc.sync.dma_start(
                        xT[:, :],
                        obss[:, t0 : t0 + TC, :].rearrange("d t b -> d (t b)"),
                    )
                stage = stpool.tile([BL, TC * A], F32, tag="stage")
                chunk_recs.append((t0, TC, stage))

                for tt in range(TC):
                    # OP-INTERLEAVED emission across the NG independent batch-
                    # group chains: engines execute their queues in order, so
                    # group-by-group emission would lockstep chain 1 a full
                    # period behind chain 0. Interleaving at op granularity
                    # keeps the stagger at ~one op, letting both chains run
                    # concurrently with per-op column counts halved.
                    # one PSUM tile per gate-bank SHARED by both groups (col
                    # halves) — PSUM is bank-granular, per-group tiles would
                    # need 8 banks and starve the decoder. Also fuses the two
                    # x-matmuls: one [128,64] start-accumulate per bank, then
                    # per-group h-matmuls stop-accumulate their column half.
                    psz_if = pzpool.tile([2 * H, BL], F32, tag="pszif", name="pszif")
                    psz_jo = pzpool.tile([2 * H, BL], F32, tag="pszjo", name="pszjo")
                    xcol = xT[:, tt * BL : (tt + 1) * BL]
                    nc.tensor.matmul(psz_if[:, :], wxif[:, :], xcol, start=True, stop=False)
                    nc.tensor.matmul(psz_jo[:, :], wxjo[:, :], xcol, start=True, stop=False)
                    gsl = [slice(g * BG, (g + 1) * BG) for g in range(NG)]
                    for g in range(NG):
                        hT = hTs[g][0 : H + 1, :]
                        nc.tensor.matmul(psz_if[:, gsl[g]], whbif, hT, start=False, stop=True)
                        nc.tensor.matmul(psz_jo[:, gsl[g]], whbjo, hT, start=False, stop=True)
                    for g in range(NG):
                        emit_dec(gt - 1, g)

                    # gate partition layout: if-bank rows = (f; i), jo-bank
                    # rows = (o; j) — f/o at base partition 0, i/j at base 64,
                    # so every 2-input DVE op pairs operands with equal bases
                    s, tj, so, tch = {}, {}, {}, {}
                    for g in range(NG):
                        s[g] = wpool.tile([2 * H, BG], FB, tag=f"s{g}", name=f"s{g}")
                        nc.scalar.activation(s[g][:, :], psz_if[:, gsl[g]], AF.Sigmoid)
                        tj_t = wpool.tile([D, BG], FB, tag=f"tj{g}")
                        tj[g] = tj_t[H : 2 * H, :]
                        nc.scalar.activation(tj[g], psz_jo[H : 2 * H, gsl[g]], AF.Tanh)
                    for g in range(NG):
                        so_t = wpool.tile([D, BG], FB, tag=f"so{g}")
                        so[g] = so_t[0:H, :]
                        nc.scalar.activation(so[g], psz_jo[0:H, gsl[g]], AF.Sigmoid)
                    for g in range(NG):
                        cst = csts[g][0:H, :]
                        cf_t = wpool.tile([D, BG], C_DT, tag=f"cf{g}")
                        cf = cf_t[0:H, :]
                        nc.vector.tensor_mul(cf, cst, s[g][0:H, :])
                        u_t = wpool.tile([D, BG], FB, tag=f"u{g}")
                        u = u_t[0:H, :]
                        nc.vector.tensor_mul(u, tj[g], s[g][H : 2 * H, :])
                        nc.vector.tensor_add(cst, cf, u)
                    for g in range(NG):
                        tch_t = wpool.tile([D, BG], FB, tag=f"tch{g}")
                        tch[g] = tch_t[0:H, :]
                        nc.scalar.activation(tch[g], csts[g][0:H, :], AF.Tanh)
                    for g in range(NG):
                        (nc.gpsimd if GP_OFFLOAD else nc.vector).tensor_mul(hTs[g][0:H, :], tch[g], so[g])
                    # decoder tanh for the block ending at gt-1, placed after
                    # this step's chain ops in the ACT queue
                    emit_dec_tanh(gt - 1)
                    gt += 1
            # finalize the last decoder block + last chunk's store
            for g in range(NG):
                emit_dec(t_total - 1, g)
            emit_dec_tanh(t_total - 1)
    nc.finalize()
    return nc


def prep_weights(lstm_kernel, lstm_bias, dec_w, dec_b):
    K = np.asarray(lstm_kernel, np.float32)
    b = np.asarray(lstm_bias, np.float32).copy()
    i_s, j_s, f_s, o_s = (slice(0, H), slice(H, 2 * H), slice(2 * H, 3 * H), slice(3 * H, 4 * H))
    b = b.copy()
    bi, bj, bf, bo = b[i_s].copy(), b[j_s].copy(), b[f_s].copy(), b[o_s].copy()
    bf += 1.0   # forget bias
    Wx, Wh = K[0:D], K[D : D + H]
    wxif = np.concatenate([Wx[:, f_s], Wx[:, i_s]], axis=1)
    wxjo = np.concatenate([Wx[:, o_s], Wx[:, j_s]], axis=1)
    whif = np.concatenate([Wh[:, f_s], Wh[:, i_s]], axis=1)
    whjo = np.concatenate([Wh[:, o_s], Wh[:, j_s]], axis=1)
    bif = np.concatenate([bf, bi])[None, :]
    bjo = np.concatenate([bo, bj])[None, :]
    whbif = np.concatenate([whif, bif], axis=0)
    whbjo = np.concatenate([whjo, bjo], axis=0)
    decwb = np.concatenate([np.asarray(dec_w, np.float32), np.asarray(dec_b, np.float32)[None, :]], axis=0)
    return (
        wxif.astype(BF16), wxjo.astype(BF16),
        whbif.astype(BF16), whbjo.astype(BF16), decwb.astype(BF16),
    )


def make_in_maps(obss, lstm_kernel, lstm_bias, dec_w, dec_b):
    wxif, wxjo, whbif, whbjo, decwb = prep_weights(lstm_kernel, lstm_bias, dec_w, dec_b)
    ob16 = np.asarray(obss).astype(BF16)
    in_maps = []
    for i in range(NCORES):
        # host-side transpose to [D, T, BL] (see build_nc comment)
        obT = np.ascontiguousarray(ob16[i * BL : (i + 1) * BL].transpose(2, 1, 0))
        in_maps.append({
            "obss": obT,
            "wxif": wxif, "wxjo": wxjo, "whbif": whbif, "whbjo": whbjo,
            "decwb": decwb,
        })
    return in_maps


def kernel(obss, lstm_kernel, lstm_bias, dec_w, dec_b, _nc_cache={}):
    if "nc" not in _nc_cache:
        _nc_cache["nc"] = build_nc()
    nc = _nc_cache["nc"]

    in_maps = make_in_maps(obss, lstm_kernel, lstm_bias, dec_w, dec_b)
    try:
        res = run_bass_kernel_spmd(nc, in_maps, core_ids=list(range(NCORES)))
    except Exception:
        # transient NRT_EXEC_UNIT_UNRECOVERABLE states clear on the next run
        res = run_bass_kernel_spmd(nc, in_maps, core_ids=list(range(NCORES)))
    outs = [res.results[i]["out"] for i in range(NCORES)]
    return np.concatenate(outs, axis=0).astype(np.float32)


if __name__ == "__main__":
    rng = np.random.default_rng(0)
    inputs = {
        "obss": rng.standard_normal((B, T, D), dtype=np.float32),
        "lstm_kernel": (rng.standard_normal((D + H, 4 * H)) * 0.1).astype(np.float32),
        "lstm_bias": np.zeros(4 * H, np.float32),
        "dec_w": (rng.standard_normal((H, A)) * 0.1).astype(np.float32),
        "dec_b": (rng.standard_normal(A) * 0.1).astype(np.float32),
    }
    out = kernel(**inputs)
    print("out", out.shape, out.dtype, out[0, 0, :4])



# revision 29
# speedup vs baseline: 1.0010x; 1.0010x over previous
"""Trainium2 Bass kernel: batch-512 LSTM (H=64, D=128, T=1024) + tanh decoder.

Strategy: data-parallel over batch across 8 NeuronCores (64 rows each).
Per core, transposed-state layout: state hT/c are [H, B] tiles; gates land in
two PSUM banks ((f,i) and (o,j)); sigmoid over the (f,i) bank starts one
matmul early, tanh(j)/sigmoid(o) follow (all in one ACT table set -> single
table load). Biases ride in via an augmented ones-row on the h-side matmul
(K=65). The per-step dependency cycle measured on silicon (~1.99us/step):
add(183) tanh_c(348) h-mul(184) whb-matmul(212) sigma_fi(314) tanh_j(310)
u-mul(188) + ~30-50ns sem gaps — every op is fixed-latency dominated, and
all cheaper reorderings were measured/modeled to be neutral or worse (see
notes: NG=2 saturates ACT at ~280ns/op engine-busy; PSUM rerouting trades
ACT access savings for equal DVE penalties).

Overhead fixes that took 2.70ms -> 2.06ms:
- decoder: per-step matmuls accumulate DEC_BLK=16 steps into a small PSUM
  tile ([64,256]f32 — a [64,512] tile caused bank pressure that slowed ALL
  engines ~20%); the batched tanh is emitted at the END of a step so it sits
  AFTER tanh_c in the in-order ACT queue (ahead of the next gate sigmoid it
  costs +690ns); global-step indexing defers each chunk's final block into
  the next chunk's first step (kills +350ns at every chunk boundary).
- obss is pre-transposed on the HOST to [D, T, BL] so the per-chunk input
  load is a fat contiguous DMA instead of an element-scattered transpose.
- chunk plan [16, 32 x 31, 16]: small first chunk -> compute starts ~4x
  sooner; small last chunk -> smaller tail store.
All recurrence elementwise runs in bf16 (end-to-end rel err ~8e-3 vs f32).
"""
import sys

sys.path.insert(0, "/opt/trn_rl_repo")

import numpy as np
import ml_dtypes

import concourse.bass as bass
import concourse.bacc as bacc
import concourse.mybir as mybir
from concourse.tile import TileContext
from concourse.bass_utils import run_bass_kernel_spmd

BF16 = ml_dtypes.bfloat16
F32 = mybir.dt.float32
FB = mybir.dt.bfloat16
AF = mybir.ActivationFunctionType
OP = mybir.AluOpType

B, T, D, H, A = 512, 1024, 128, 64, 16
NCORES = 8
BL = B // NCORES  # 64 batch rows per core
TC = 32           # timesteps per chunk (small first chunk -> fast start)
DEC_BLK = 8       # timesteps per decoder PSUM bank (8*16 = 128 f32; the
                  # [64,256] tanh slightly overflowed the post-tanh_c ACT
                  # window, +115ns on ~20 steps)

C_DT = FB         # cell-state dtype (bf16 verified: end-to-end rel err ~8e-3)
NG = 1            # interleaved batch groups per core (NG=2 saturates ACT:
                  # ~280ns engine-busy per ACT op regardless of columns, and
                  # 8 gate-ACTs/step push the scalar engine to 93% -> 2.84ms)
GP_OFFLOAD = False # run m1/v4 on GpSimd to unload the vector engine


def build_nc(t_total=T):
    nc = bacc.Bacc()
    # obss arrives HOST-PRE-TRANSPOSED as [D, T, BL]: the per-chunk load is
    # then a fat contiguous DMA (4KB/partition) instead of an element-
    # scattered dma_start_transpose that kept the DMA engines busy >50% of
    # the run and contended with ACT/DVE SBUF ports.
    obss = nc.declare_dram_parameter("obss", [D, T, BL], FB, isOutput=False)
    wxif_d = nc.declare_dram_parameter("wxif", [D, 2 * H], FB, isOutput=False)
    wxjo_d = nc.declare_dram_parameter("wxjo", [D, 2 * H], FB, isOutput=False)
    whbif_d = nc.declare_dram_parameter("whbif", [H + 1, 2 * H], FB, isOutput=False)
    whbjo_d = nc.declare_dram_parameter("whbjo", [H + 1, 2 * H], FB, isOutput=False)
    decwb_d = nc.declare_dram_parameter("decwb", [H + 1, A], FB, isOutput=False)
    out = nc.declare_dram_parameter("out", [BL, T, A], F32, isOutput=True)

    with TileContext(nc) as tc:
        with (
            tc.tile_pool(name="const", bufs=1) as cpool,
            tc.tile_pool(name="state", bufs=1) as spool,
            tc.tile_pool(name="xT", bufs=2) as xpool,
            tc.tile_pool(name="stage", bufs=2) as stpool,
            tc.tile_pool(name="work", bufs=3) as wpool,
            tc.tile_pool(name="psz", bufs=2, space="PSUM") as pzpool,
            tc.tile_pool(name="psd", bufs=2, space="PSUM") as pdpool,
        ):
            # all tiles allocated 128-partition so every base partition is 0
            # (2-input DVE ops require equal input base partitions)
            wxif = cpool.tile([D, 2 * H], FB, tag="wxif")
            wxjo = cpool.tile([D, 2 * H], FB, tag="wxjo")
            whbif_t = cpool.tile([D, 2 * H], FB, tag="whbif")
            whbjo_t = cpool.tile([D, 2 * H], FB, tag="whbjo")
            decwb_t = cpool.tile([D, A], FB, tag="decwb")
            whbif = whbif_t[0 : H + 1, :]
            whbjo = whbjo_t[0 : H + 1, :]
            decwb = decwb_t[0 : H + 1, :]
            # first input chunk's DMA is hoisted AHEAD of the weight loads:
            # the Sync queue generates descriptors serially (~0.7us each), and
            # the first matmul needs chunk 0's 256KB before anything happens —
            # queueing it first starts compute ~3.5us sooner.
            FIRST_TC = 16
            xT0 = xpool.tile([D, FIRST_TC * BL], FB, tag="xT0", name="xT0")
            nc.sync.dma_start(
                xT0[:, :],
                obss[:, 0:FIRST_TC, :].rearrange("d t b -> d (t b)"),
            )
            # weight loads on the Sync queue after chunk 0's input (moving
            # them to the ACT queue measured slightly worse: 2055184 vs
            # 2051350)
            nc.sync.dma_start(wxif[:, :], wxif_d[:, :])
            nc.sync.dma_start(whbif, whbif_d[:, :])
            nc.sync.dma_start(wxjo[:, :], wxjo_d[:, :])
            nc.sync.dma_start(whbjo, whbjo_d[:, :])
            nc.sync.dma_start(decwb, decwb_d[:, :])

            BG = BL // NG  # batch rows per group
            hTs, csts = [], []
            for g in range(NG):
                hT_t = spool.tile([D, BG], FB, tag=f"hT{g}")
                cst_t = spool.tile([D, BG], C_DT, tag=f"c{g}")
                nc.vector.memset(hT_t[0:H, :], 0.0)
                nc.vector.memset(hT_t[H : H + 1, :], 1.0)
                nc.vector.memset(cst_t[0:H, :], 0.0)
                hTs.append(hT_t)
                csts.append(cst_t)

            # chunk plan: small first chunk -> first input DMA lands sooner;
            # tiny last chunks -> the only output DMA that can't overlap
            # compute (the final one) shrinks to 64KB
            chunk_plan = [16] + [32] * ((t_total - 32) // 32) + [8, 8]
            assert sum(chunk_plan) == t_total
            dec_state = {}
            chunk_recs = []  # (t0, tc, stage tile) in emission order

            def stage_of(gt):
                for t0, tc_, stg in chunk_recs:
                    if t0 <= gt < t0 + tc_:
                        return t0, tc_, stg
                raise AssertionError(gt)

            def emit_dec(gt, g):
                # decoder matmul for GLOBAL step gt, group g:
                # out[b, A] = h @ dec_w + dec_b via the ones-row of hT
                # (batched tanh emitted separately, off the ACT critical path)
                if gt < 0:
                    return
                dcol = gt % DEC_BLK
                if dcol == 0 and g == 0:
                    psd_tile = pdpool.tile([BL, DEC_BLK * A], F32, tag="psd")
                    dec_state["psd"] = psd_tile
                psd = dec_state["psd"]
                nc.tensor.matmul(
                    psd[g * BG : (g + 1) * BG, dcol * A : (dcol + 1) * A],
                    hTs[g][0 : H + 1, :], decwb, start=True, stop=True,
                )

            def emit_dec_tanh(gt):
                # batched decoder tanh for the DEC_BLK block ending at global
                # step gt. Emitted at the END of a step's ACT queue so it
                # fills the ACT idle window after tanh(c) instead of delaying
                # the next step's gate sigmoid (+690ns per occurrence). Global
                # indexing defers a chunk's final block into the next chunk's
                # first step (fixes a +350ns stall at every chunk boundary);
                # the chunk's output DMA is emitted right after it.
                if gt < 0 or (gt % DEC_BLK) != DEC_BLK - 1:
                    return
                t0, tc_, stg = stage_of(gt)
                blk0 = ((gt - t0) // DEC_BLK) * DEC_BLK
                nc.scalar.activation(
                    stg[:, blk0 * A : (blk0 + DEC_BLK) * A],
                    dec_state["psd"][:, :], AF.Tanh,
                )
                if gt == t0 + tc_ - 1:
                    nc.sync.dma_start(out[:, t0 : t0 + tc_, :], stg[:, :])

            gt = 0
            for ci, TC in enumerate(chunk_plan):
                t0 = gt
                if ci == 0:
                    assert TC == FIRST_TC
                    xT = xT0
                else:
                    xT = xpool.tile([D, TC * BL], FB, tag="xT")
                    nc.sync.dma_start(
                        xT[:, :],
                        obss[:, t0 : t0 + TC, :].rearrange("d t b -> d (t b)"),
                    )
                stage = stpool.tile([BL, TC * A], F32, tag="stage")
                chunk_recs.append((t0, TC, stage))

                for tt in range(TC):
                    # OP-INTERLEAVED emission across the NG independent batch-
                    # group chains: engines execute their queues in order, so
                    # group-by-group emission would lockstep chain 1 a full
                    # period behind chain 0. Interleaving at op granularity
                    # keeps the stagger at ~one op, letting both chains run
                    # concurrently with per-op column counts halved.
                    # one PSUM tile per gate-bank SHARED by both groups (col
                    # halves) — PSUM is bank-granular, per-group tiles would
                    # need 8 banks and starve the decoder. Also fuses the two
                    # x-matmuls: one [128,64] start-accumulate per bank, then
                    # per-group h-matmuls stop-accumulate their column half.
                    psz_if = pzpool.tile([2 * H, BL], F32, tag="pszif", name="pszif")
                    psz_jo = pzpool.tile([2 * H, BL], F32, tag="pszjo", name="pszjo")
                    xcol = xT[:, tt * BL : (tt + 1) * BL]
                    nc.tensor.matmul(psz_if[:, :], wxif[:, :], xcol, start=True, stop=False)
                    nc.tensor.matmul(psz_jo[:, :], wxjo[:, :], xcol, start=True, stop=False)
                    gsl = [slice(g * BG, (g + 1) * BG) for g in range(NG)]
                    for g in range(NG):
                        hT = hTs[g][0 : H + 1, :]
                        nc.tensor.matmul(psz_if[:, gsl[g]], whbif, hT, start=False, stop=True)
                        nc.tensor.matmul(psz_jo[:, gsl[g]], whbjo, hT, start=False, stop=True)
                    for g in range(NG):
                        emit_dec(gt - 1, g)

                    # gate partition layout: if-bank rows = (f; i), jo-bank
                    # rows = (o; j) — f/o at base partition 0, i/j at base 64,
                    # so every 2-input DVE op pairs operands with equal bases
                    s, tj, so, tch = {}, {}, {}, {}
                    for g in range(NG):
                        s[g] = wpool.tile([2 * H, BG], FB, tag=f"s{g}", name=f"s{g}")
                        nc.scalar.activation(s[g][:, :], psz_if[:, gsl[g]], AF.Sigmoid)
                        tj_t = wpool.tile([D, BG], FB, tag=f"tj{g}")
                        tj[g] = tj_t[H : 2 * H, :]
                        nc.scalar.activation(tj[g], psz_jo[H : 2 * H, gsl[g]], AF.Tanh)
                    for g in range(NG):
                        so_t = wpool.tile([D, BG], FB, tag=f"so{g}")
                        so[g] = so_t[0:H, :]
                        nc.scalar.activation(so[g], psz_jo[0:H, gsl[g]], AF.Sigmoid)
                    for g in range(NG):
                        cst = csts[g][0:H, :]
                        cf_t = wpool.tile([D, BG], C_DT, tag=f"cf{g}")
                        cf = cf_t[0:H, :]
                        nc.vector.tensor_mul(cf, cst, s[g][0:H, :])
                        u_t = wpool.tile([D, BG], FB, tag=f"u{g}")
                        u = u_t[0:H, :]
                        nc.vector.tensor_mul(u, tj[g], s[g][H : 2 * H, :])
                        nc.vector.tensor_add(cst, cf, u)
                    for g in range(NG):
                        tch_t = wpool.tile([D, BG], FB, tag=f"tch{g}")
                        tch[g] = tch_t[0:H, :]
                        nc.scalar.activation(tch[g], csts[g][0:H, :], AF.Tanh)
                    for g in range(NG):
                        (nc.gpsimd if GP_OFFLOAD else nc.vector).tensor_mul(hTs[g][0:H, :], tch[g], so[g])
                    # decoder tanh for the block ending at gt-1, placed after
                    # this step's chain ops in the ACT queue
                    emit_dec_tanh(gt - 1)
                    gt += 1
            # finalize the last decoder block + last chunk's store
            for g in range(NG):
                emit_dec(t_total - 1, g)
            emit_dec_tanh(t_total - 1)
    nc.finalize()
    return nc


def prep_weights(lstm_kernel, lstm_bias, dec_w, dec_b):
    K = np.asarray(lstm_kernel, np.float32)
    b = np.asarray(lstm_bias, np.float32).copy()
    i_s, j_s, f_s, o_s = (slice(0, H), slice(H, 2 * H), slice(2 * H, 3 * H), slice(3 * H, 4 * H))
    b = b.copy()
    bi, bj, bf, bo = b[i_s].copy(), b[j_s].copy(), b[f_s].copy(), b[o_s].copy()
    bf += 1.0   # forget bias
    Wx, Wh = K[0:D], K[D : D + H]
    wxif = np.concatenate([Wx[:, f_s], Wx[:, i_s]], axis=1)
    wxjo = np.concatenate([Wx[:, o_s], Wx[:, j_s]], axis=1)
    whif = np.concatenate([Wh[:, f_s], Wh[:, i_s]], axis=1)
    whjo = np.concatenate([Wh[:, o_s], Wh[:, j_s]], axis=1)
    bif = np.concatenate([bf, bi])[None, :]
    bjo = np.concatenate([bo, bj])[None, :]
    whbif = np.concatenate([whif, bif], axis=0)
    whbjo = np.concatenate([whjo, bjo], axis=0)
    decwb = np.concatenate([np.asarray(dec_w, np.float32), np.asarray(dec_b, np.float32)[None, :]], axis=0)
    return (
        wxif.astype(BF16), wxjo.astype(BF16),
        whbif.astype(BF16), whbjo.astype(BF16), decwb.astype(BF16),
    )


def make_in_maps(obss, lstm_kernel, lstm_bias, dec_w, dec_b):
    wxif, wxjo, whbif, whbjo, decwb = prep_weights(lstm_kernel, lstm_bias, dec_w, dec_b)
    ob16 = np.asarray(obss).astype(BF16)
    in_maps = []
    for i in range(NCORES):
        # host-side transpose to [D, T, BL] (see build_nc comment)
        obT = np.ascontiguousarray(ob16[i * BL : (i + 1) * BL].transpose(2, 1, 0))
        in_maps.append({
            "obss": obT,
            "wxif": wxif, "wxjo": wxjo, "whbif": whbif, "whbjo": whbjo,
            "decwb": decwb,
        })
    return in_maps


def kernel(obss, lstm_kernel, lstm_bias, dec_w, dec_b, _nc_cache={}):
    if "nc" not in _nc_cache:
        _nc_cache["nc"] = build_nc()
    nc = _nc_cache["nc"]

    in_maps = make_in_maps(obss, lstm_kernel, lstm_bias, dec_w, dec_b)
    try:
        res = run_bass_kernel_spmd(nc, in_maps, core_ids=list(range(NCORES)))
    except Exception:
        # transient NRT_EXEC_UNIT_UNRECOVERABLE states clear on the next run
        res = run_bass_kernel_spmd(nc, in_maps, core_ids=list(range(NCORES)))
    outs = [res.results[i]["out"] for i in range(NCORES)]
    return np.concatenate(outs, axis=0).astype(np.float32)


if __name__ == "__main__":
    rng = np.random.default_rng(0)
    inputs = {
        "obss": rng.standard_normal((B, T, D), dtype=np.float32),
        "lstm_kernel": (rng.standard_normal((D + H, 4 * H)) * 0.1).astype(np.float32),
        "lstm_bias": np.zeros(4 * H, np.float32),
        "dec_w": (rng.standard_normal((H, A)) * 0.1).astype(np.float32),
        "dec_b": (rng.standard_normal(A) * 0.1).astype(np.float32),
    }
    out = kernel(**inputs)
    print("out", out.shape, out.dtype, out[0, 0, :4])

